# revision 32
# baseline (speedup 1.0000x reference)
"""MixtureLinear Trainium2 kernel.

Computes, for B=256, IN=1024, OUT=1024, RANK=16:
    out[b,o] = sum_i input[b,i] * sum_r weight[o,i,r] * coef[b,r]
             + sum_r bias[o,r] * coef[b,r]

Strategy (8 NeuronCores, tensor-parallel on OUT):
  - Core c owns OUT rows [128c, 128c+128). It reads only its weight shard
    (1/8 of the 64MB weight tensor), input/coef replicated.
  - Stage 1 (PE): proj[b,(o,r)] = inputT.T @ W2 where W2[i, o*16+r] =
    weight[o,i,r]; K=IN accumulated over 8 psum matmuls per 512-column
    chunk (one psum bank, 32 o's x 16 r's per chunk).
  - Stage 2 (DVE): out[b,o] = sum_r proj[b,(o,r)] * coef[b,r] via a
    broadcast-AP multiply + strided reduce over the innermost rank axis.
  - Bias: one tiny K=16 matmul per b-chunk: coefT.T @ biasT -> psum,
    added in the final DVE add before the output DMA.

Matmul dtype is selectable via MIXL_DT (float16 default; bfloat16 /
float32r / float32 supported). Host pre-casts and pre-transposes shards;
stage-2 and all accumulation stay fp32.
"""

import os
import sys
from contextlib import ExitStack

sys.path.insert(0, "/opt/trn_rl_repo")

import numpy as np
import ml_dtypes

import concourse.bass as bass
import concourse.tile as tile
from concourse import bacc, mybir
from concourse.bass_utils import run_bass_kernel_spmd

B, IN, OUT, RANK = 256, 1024, 1024, 16
NCORES = 8
OUTL = OUT // NCORES        # 128 out rows per core
P = 128                     # partitions
NB = B // P                 # 2 batch chunks
NK = IN // P                # 8 contraction chunks
CH = 512                    # psum chunk: one fp32 bank
NCH = OUTL * RANK // CH     # 4 column chunks per core
OCH = CH // RANK            # 32 o's per chunk

DT_NAME = os.environ.get("MIXL_DT", "float16")
IMPL = os.environ.get("MIXL_IMPL", "v2")
N_WARM1 = int(os.environ.get("MIXL_W1", "22"))
N_WARM2 = int(os.environ.get("MIXL_W2", "0"))
FP8 = os.environ.get("MIXL_FP8", "0") == "1"
NK16 = 6 if FP8 else NK      # k-chunks carried in fp16

_DT_MAP = {
    "float16": (mybir.dt.float16, np.float16),
    "bfloat16": (mybir.dt.bfloat16, ml_dtypes.bfloat16),
    "float32r": (mybir.dt.float32r, np.float32),
    "float32": (mybir.dt.float32, np.float32),
}

_SEMMAX = os.environ.get("MIXL_SEMMAX")
if _SEMMAX:
    # Experiment: cap walrus's semaphore allocation; if its exit-sequence
    # zero-storm length follows the cap, the ~6.8us exit shrinks.
    from concourse import bass_utils as _bu
    _orig_gwa = _bu.get_walrus_args

    def _gwa(*a, **kw):
        return _orig_gwa(*a, **kw) + [f"--max-sem-num={_SEMMAX}"]

    _bu.get_walrus_args = _gwa


class _NoBarrierBlock(bass.BassBlock):
    """BassBlock without the exit drain + all-engine barrier.

    The NRT epilogue (per-engine semaphore-zero storm + exit rendezvous
    chain, ~7us total) runs after each engine's stream ends. With the
    stock barrier, every engine waits for the slowest one before starting
    its epilogue share; without it, early-finishing engines overlap their
    epilogue with the critical-path tail. Output completion is still
    guaranteed: gpsimd's terminal s_out wait orders NEFF completion after
    the output DMAs.
    """

    def __exit__(self, exc_type, exc_val, exc_tb):
        if exc_type is not None:
            return
        for engine, last_body in self.last_body.items():
            with self.bass.body(
                last_body, parent=self.bass.cur_bb, allow_existing_parent=True
            ):
                engine.br(self.end_bb)
        self.bass.switch_bb(self.end_bb)


def build_nc_raw(dt_name=DT_NAME):
    """Raw-Bass (manual Block + semaphores) implementation.

    Design notes (all HW-measured on this container):
    - The NRT execution envelope (entry rendezvous chain + per-engine
      instruction-table loads at the front; per-engine semaphore-zero
      storm + exit rendezvous at the back) costs ~14us on an empty
      kernel. The exit part runs after each ENGINE's stream ends, so the
      Block-end all-engine barrier is deliberately skipped (engines that
      finish early overlap their epilogue with the critical-path tail).
    - A single DMA transfer only sustains ~100-200 GB/s; aggregate tops
      out ~430 GB/s, and the SDMA engines round-robin across everything
      queued at packet granularity. So loads go out in consumption order
      as ~10 chunks with a sliding window of 3 in flight, with tiny
      first chunks so the first matmul can start ASAP.
    - 8 dummy matmuls on memset data warm the PE HAM clock (1.2->2.4GHz
      after ~3.4us of sustained activity) while the first loads land.
    - LDWEIGHTS is emitted separately from a non-self-loading Matmult
      (inst.ldweights=False) so weight loads pipeline into the PE's
      background buffer; fused matmuls measured ~600ns vs ~380ns split.
    """
    dt, _ = _DT_MAP[dt_name]
    f32 = mybir.dt.float32
    f16 = mybir.dt.float16
    nc = bacc.Bacc("TRN2", target_bir_lowering=False, debug=False)

    xT = nc.declare_dram_parameter("xT", [IN, B], dt, isOutput=False)
    w2 = nc.declare_dram_parameter("w2", [NCH, P, NK * CH], dt, isOutput=False)
    coef = nc.declare_dram_parameter("coef", [B, RANK], f32, isOutput=False)
    coefT = nc.declare_dram_parameter("coefT", [RANK, B], dt, isOutput=False)
    biasT = nc.declare_dram_parameter("biasT", [RANK, OUTL], dt, isOutput=False)
    out = nc.declare_dram_parameter("out", [B, OUTL], f32, isOutput=True)

    w2v = w2.rearrange("n p (k c) -> n p k c", c=CH)
    xTv = xT.rearrange("(k p) b -> p k b", p=P)
    coefv = coef.rearrange("(nb p) r -> p nb r", p=P)

    with ExitStack() as ctx:
        sb = lambda shape, d, name: ctx.enter_context(
            nc.sbuf_tensor(name, shape, d))
        xT_t = sb([P, NK, B], dt, "xT_t")
        wts = [sb([P, NK, CH], dt, f"wt{n}") for n in range(NCH)]
        coef_t = sb([P, NB, RANK], f32, "coef_t")
        coefT_t = sb([RANK, B], dt, "coefT_t")
        biasT_t = sb([RANK, OUTL], dt, "biasT_t")
        warm_t = sb([P, CH], dt, "warm_t")
        tmps = [sb([P, OCH, RANK], f16, f"tmp{i}") for i in range(2)]
        out_sb = [sb([P, OUTL], f32, f"osum{b}") for b in range(NB)]
        outf = [sb([P, OUTL], f32, f"outf{b}") for b in range(NB)]
        pss = [ctx.enter_context(nc.psum_tensor(f"ps{g}", [P, CH], f32))
               for g in range(8)]

        # One semaphore per DMA: +16 increments from different transfers
        # interleave (per-SDMA-engine +1s), so aggregate thresholds on a
        # shared sem do not prove any single transfer completed.
        nsem = lambda name: ctx.enter_context(nc.semaphore(name))
        s_x0 = nsem("s_x0")        # xT k=0 (64KB)
        s_w00 = nsem("s_w00")      # w n0 k=0 (128KB)
        s_x13 = nsem("s_x13")      # xT k=1..3
        s_w013 = nsem("s_w013")    # w n0 k=1..3
        s_x47 = nsem("s_x47")      # xT k=4..7
        s_w047 = nsem("s_w047")    # w n0 k=4..7
        s_wn = [nsem(f"s_wn{n}") for n in range(1, NCH)]   # w1..w3 (1MB)
        s_gc = nsem("s_gc")        # coefT
        s_gb = nsem("s_gb")        # biasT
        s_gf = nsem("s_gf")        # coef (fp32)
        s_warm = nsem("s_warm")    # warm-up tile memset
        s_pe = nsem("s_pe")        # psum groups done
        s_dvm = nsem("s_dvm")      # psum mults done
        s_red = nsem("s_red")      # reduces done
        s_dve = nsem("s_dve")      # outf ready
        s_out = nsem("s_out")      # output DMA done

        with _NoBarrierBlock(nc, f"block_{nc.next_id()}") as block:

            @block.sync
            def _(sync):
                xfers = [
                    (xT_t[:, 0:1, :], xTv[:, 0:1, :], s_x0),
                    (wts[0][:, 0:1, :], w2v[0][:, 0:1, :], s_w00),
                    (xT_t[:, 1:4, :], xTv[:, 1:4, :], s_x13),
                    (wts[0][:, 1:4, :], w2v[0][:, 1:4, :], s_w013),
                    (xT_t[:, 4:, :], xTv[:, 4:, :], s_x47),
                    (wts[0][:, 4:, :], w2v[0][:, 4:, :], s_w047),
                ] + [(wts[n][:], w2v[n], s_wn[n - 1]) for n in range(1, NCH)]
                for i, (dst, srcv, sem) in enumerate(xfers):
                    if i >= 3:
                        sync.wait_ge(xfers[i - 3][2], 16)
                    sync.dma_start(dst, srcv).then_inc(sem, 16)

            @block.scalar
            def _(scalar):
                # Output DMAs on the (otherwise idle) ACT ring.
                for b in range(NB):
                    scalar.wait_ge(s_dve, b + 1)
                    scalar.dma_start(out[b * P:(b + 1) * P, :],
                                     outf[b][:]).then_inc(s_out, 16)

            @block.gpsimd
            def _(gpsimd):
                gpsimd.memset(warm_t[:], 0.25).then_inc(s_warm, 1)
                gpsimd.dma_start(coef_t[:], coefv).then_inc(s_gf, 16)
                gpsimd.dma_start(coefT_t[:], coefT[:]).then_inc(s_gc, 16)
                gpsimd.dma_start(biasT_t[:], biasT[:]).then_inc(s_gb, 16)
                # Terminal waiter: holds the Pool stream until outputs are
                # in DRAM, so NEFF completion implies outputs landed.
                gpsimd.wait_ge(s_out, 32)

            @block.tensor
            def _(pe):
                # HAM warm-up: ~8 dummy matmuls on memset data while the
                # first real chunks are still in flight.
                pe.wait_ge(s_warm, 1)
                for _i in range(8):
                    nc.tensor.matmul(pss[2][:], lhsT=warm_t[:, 0:P],
                                     rhs=warm_t[:], start=True, stop=True)
                for n in range(NCH):
                    bank = (2 * n) % 6
                    for k in range(NK):
                        if n == 0:
                            if k == 0:
                                pe.wait_ge(s_x0, 16)
                                pe.wait_ge(s_w00, 16)
                            elif k == 1:
                                pe.wait_ge(s_x13, 16)
                                pe.wait_ge(s_w013, 16)
                            elif k == 4:
                                pe.wait_ge(s_x47, 16)
                                pe.wait_ge(s_w047, 16)
                        elif k == 0:
                            pe.wait_ge(s_wn[n - 1], 16)
                        if n == 3 and k == 0:
                            # banks 0/1 reused: n0 multiplies must be done
                            pe.wait_ge(s_dvm, 2)
                        for b in range(NB):
                            # split LDWEIGHTS + non-self-loading matmul
                            nc.tensor.ldweights(xT_t[:, k, b * P:(b + 1) * P])
                            mm = nc.tensor.matmul(
                                pss[bank + b][:],
                                lhsT=xT_t[:, k, b * P:(b + 1) * P],
                                rhs=wts[n][:, k, :],
                                start=(k == 0),
                                stop=(k == NK - 1),
                            )
                            mm.ins.ldweights = False
                            if k == NK - 1:
                                mm.then_inc(s_pe, 1)
                    if n == 0:
                        # Bias matmuls into dedicated banks 6/7, slotted here
                        # so their input DMAs are long done and the PE stream
                        # never stalls on them.
                        pe.wait_ge(s_gc, 16)
                        pe.wait_ge(s_gb, 16)
                        for b in range(NB):
                            nc.tensor.matmul(
                                pss[6 + b][:, 0:OUTL],
                                lhsT=coefT_t[:, b * P:(b + 1) * P],
                                rhs=biasT_t[:],
                                start=True, stop=True,
                            ).then_inc(s_pe, 1)

            @block.vector
            def _(vector):
                vector.wait_ge(s_gf, 16)
                # s_pe increment order: n0b0=1 n0b1=2 bias0=3 bias1=4
                # n1b0=5 n1b1=6 n2b0=7 n2b1=8 n3b0=9 n3b1=10
                pe_val = {0: (1, 2), 1: (5, 6), 2: (7, 8), 3: (9, 10)}
                g = 0
                for n in range(NCH):
                    bank = (2 * n) % 6
                    for b in range(NB):
                        g += 1
                        vector.wait_ge(s_pe, pe_val[n][b])
                        coef_b = coef_t[:, b, :].rearrange(
                            "p (one r) -> p one r", one=1)
                        tmp = tmps[g % 2]
                        nc.vector.tensor_mul(
                            tmp[:],
                            pss[bank + b][:].rearrange("p (o r) -> p o r", r=RANK),
                            coef_b.to_broadcast((P, OCH, RANK)),
                        ).then_inc(s_dvm, 1)
                        vector.wait_ge(s_dvm, g)
                        nc.vector.tensor_reduce(
                            out_sb[b][:, n * OCH:(n + 1) * OCH],
                            tmp[:],
                            axis=mybir.AxisListType.X,
                            op=mybir.AluOpType.add,
                        ).then_inc(s_red, 1)
                for b in range(NB):
                    vector.wait_ge(s_pe, 3 + b)
                    # all four reduces of this b-chunk (g = b+1, b+3, b+5, b+7)
                    vector.wait_ge(s_red, NB * NCH - NB + b + 1)
                    nc.vector.tensor_add(
                        outf[b][:], out_sb[b][:], pss[6 + b][:, 0:OUTL]
                    ).then_inc(s_dve, 1)

    nc.compile()
    return nc


def build_nc_v2(dt_name=DT_NAME, n_warm1=None, n_warm2=None):
    """v2 raw-Bass implementation, rebuilt from baseline trace analysis.

    Trace findings the baseline left on the table:
    - Steady-state matmuls issue every 216ns (512 cols @ 2.4GHz) — the PE
      pipe itself is perfect. But two ~2us weight-DMA stalls early in the
      stream made the HAM drop the PE clock to 1.2GHz for ~7us.
    - All big loads went through ONE hw queue (sync) at an effective
      ~160GB/s early / 276GB/s late; per-transfer completion order plus a
      window-of-3 starved the leading edge.
    - The stage-2 (rank-contract) chain ran entirely on DVE, serializing
      ~3.1us after the last matmul.
    - Exit = rendezvous + 256-semaphore zero storm (~7us, walrus-fixed).

    v2 changes:
    - Weights stream as 16 x 256KB transfers (k-pair granularity)
      alternated across BOTH hw queues (sync + scalar) in consumption
      order; x as 4 x 128KB split across both; no windowing — hw queues
      process descriptors in order. coef (fp32, stage-2) rides SWDGE.
    - PE: 8 psum banks = 8 (n,b) accumulation groups, no bank reuse, no
      mid-stream back-pressure waits. Chunk waits land one-per-instruction
      (x-waits on ldweights, w-waits on matmuls) so they cost nothing.
    - Bias matmuls run right after warmup into banks 6/7 (n3's banks);
      gpsimd copies them to SBUF long before n3's k0 start=True wipe.
    - Stage-2 split: gpsimd owns b0's mult+reduce+add chain, vector owns
      b1's. Within an engine program order replaces semaphores.
    - Output DMAs on scalar (its weight share is long done); gpsimd's
      terminal s_out wait still orders NEFF completion after the output.
    """
    if n_warm1 is None:
        n_warm1 = N_WARM1
    if n_warm2 is None:
        n_warm2 = N_WARM2
    dt, _ = _DT_MAP[dt_name]
    f32 = mybir.dt.float32
    f16 = mybir.dt.float16
    nc = bacc.Bacc("TRN2", target_bir_lowering=False, debug=False)

    xT = nc.declare_dram_parameter("xT", [IN, B], dt, isOutput=False)
    w2 = nc.declare_dram_parameter("w2", [NCH, P, NK * CH], dt, isOutput=False)
    cb = nc.declare_dram_parameter("cb", [RANK, B + OUTL], dt, isOutput=False)
    coef = nc.declare_dram_parameter("coef", [B, RANK], f32, isOutput=False)
    out = nc.declare_dram_parameter("out", [B, OUTL], f32, isOutput=True)

    w2v = w2.rearrange("n p (k c) -> n p k c", c=CH)
    xTv = xT.rearrange("(k p) b -> p k b", p=P)
    coefv = coef.rearrange("(nb p) r -> p nb r", p=P)

    with ExitStack() as ctx:
        sb = lambda shape, d, name: ctx.enter_context(
            nc.sbuf_tensor(name, shape, d))
        xT_t = sb([P, NK, B], dt, "xT_t")
        wts = [sb([P, NK, CH], dt, f"wt{n}") for n in range(NCH)]
        cb_t = sb([RANK, B + OUTL], dt, "cb_t")
        coef_t = sb([P, NB, RANK], f32, "coef_t")
        warm_t = sb([P, CH], dt, "warm_t")
        tmp_g = [sb([P, OCH, RANK], f16, f"tmp_g{i}") for i in range(2)]
        tmp_v = sb([P, OCH, RANK], f16, "tmp_v")
        bias_sb = [sb([P, OUTL], f32, f"bias_sb{b}") for b in range(NB)]
        out_sb = [sb([P, OUTL], f32, f"osum{b}") for b in range(NB)]
        outf = [sb([P, OUTL], f32, f"outf{b}") for b in range(NB)]
        pss = [ctx.enter_context(nc.psum_tensor(f"ps{g}", [P, CH], f32))
               for g in range(8)]

        nsem = lambda name: ctx.enter_context(nc.semaphore(name))
        s_warm = nsem("s_warm")
        s_cb = nsem("s_cb")
        s_cf = nsem("s_cf")
        s_biasmm = nsem("s_biasmm")
        s_bcopy = nsem("s_bcopy")
        s_x = [nsem(f"s_x{i}") for i in range(4)]          # x k-pairs
        s_w = [[nsem(f"s_w{n}_{kk}") for kk in range(4)]   # w (n, k-pair)
               for n in range(NCH)]
        s_pe = nsem("s_pe")
        s_r0 = nsem("s_r0")      # vector b0 reduces done
        s_addg = nsem("s_addg")
        s_addv = nsem("s_addv")
        s_out = nsem("s_out")

        with _NoBarrierBlock(nc, f"block_{nc.next_id()}") as block:

            def wxfer(eng, n, kk):
                eng.dma_start(
                    wts[n][:, 2 * kk:2 * kk + 2, :],
                    w2v[n][:, 2 * kk:2 * kk + 2, :],
                ).then_inc(s_w[n][kk], 16)

            def xxfer(eng, kk):
                eng.dma_start(
                    xT_t[:, 2 * kk:2 * kk + 2, :],
                    xTv[:, 2 * kk:2 * kk + 2, :],
                ).then_inc(s_x[kk], 16)

            @block.sync
            def _(sync):
                # The 16-SDMA pool is shared across queues (~320GB/s total,
                # ~160 each when both stream); transfers are laid out so the
                # two queues jointly deliver in PE consumption order.
                wxfer(sync, 0, 0)      # w n0 k01   need s
                xxfer(sync, 1)         # x k23      need s+0.86
                wxfer(sync, 0, 2)      # w n0 k45   need s+1.73
                xxfer(sync, 3)         # x k67      need s+2.6
                for n in range(1, NCH):
                    wxfer(sync, n, 0)
                    wxfer(sync, n, 2)
                # Output b1 rides the (long-idle) sync queue, parallel with
                # scalar's b0 output issue.
                sync.wait_ge(s_addv, 1)
                sync.dma_start(out[P:2 * P, :],
                               outf[1][:]).then_inc(s_out, 16)

            @block.scalar
            def _(scalar):
                scalar.dma_start(cb_t[:], cb[:]).then_inc(s_cb, 16)
                xxfer(scalar, 0)       # x k01      need s
                wxfer(scalar, 0, 1)    # w n0 k23   need s+0.86
                xxfer(scalar, 2)       # x k45      need s+1.73
                wxfer(scalar, 0, 3)    # w n0 k67   need s+2.6
                for n in range(1, NCH):
                    wxfer(scalar, n, 1)
                    wxfer(scalar, n, 3)
                # Bias psum -> SBUF (gpsimd can't read PSUM; scalar can).
                scalar.wait_ge(s_biasmm, 2)
                for b in range(NB):
                    nc.scalar.copy(
                        bias_sb[b][:], pss[6 + b][:, 0:OUTL]
                    ).then_inc(s_bcopy, 1)
                scalar.wait_ge(s_addg, 1)
                scalar.dma_start(out[0:P, :],
                                 outf[0][:]).then_inc(s_out, 16)

            @block.tensor
            def _(pe):
                # Pure warmup until the first weight chunks are visible; any
                # mid-warmup stall would reset the HAM clock ramp (3us of
                # continuous execution -> 2.4GHz), so nothing here waits on
                # DMA. Bias matmuls run at the n0/n1 seam instead, inside
                # the full-speed stream (cb lands long before).
                pe.wait_ge(s_warm, 1)
                for _i in range(n_warm1 + n_warm2):
                    nc.tensor.matmul(pss[0][:], lhsT=warm_t[:, 0:P],
                                     rhs=warm_t[:], start=True, stop=True)
                # n0-n2: b-interleaved k-loop (spreads the x-arrival deadline
                # over the full n0 phase). n3: b0 k-sweep then b1 k-sweep, so
                # b0's last chunk closes 1.73us before b1's and the stage-2
                # tail pipelines into the matmul stream.
                for n in range(NCH):
                    if n == 1:
                        # Bias matmuls into banks 6/7 (n3's banks, reused
                        # much later); ~300ns each inside the hot stream.
                        pe.wait_ge(s_cb, 16)
                        for b in range(NB):
                            nc.tensor.matmul(
                                pss[6 + b][:, 0:OUTL],
                                lhsT=cb_t[:, b * P:(b + 1) * P],
                                rhs=cb_t[:, B:B + OUTL],
                                start=True, stop=True,
                            ).then_inc(s_biasmm, 1)
                    if n < NCH - 1:
                        kb = [(k, b) for k in range(NK) for b in range(NB)]
                    else:
                        kb = [(k, b) for b in range(NB) for k in range(NK)]
                    for k, b in kb:
                        if b == 0:
                            if n == 0 and k % 2 == 0:
                                pe.wait_ge(s_x[k // 2], 16)
                            if n == 3 and k == 0:
                                pe.wait_ge(s_bcopy, 2)
                        nc.tensor.ldweights(
                            xT_t[:, k, b * P:(b + 1) * P])
                        if b == 0 and k % 2 == 0:
                            pe.wait_ge(s_w[n][k // 2], 16)
                        mm = nc.tensor.matmul(
                            pss[2 * n + b][:],
                            lhsT=xT_t[:, k, b * P:(b + 1) * P],
                            rhs=wts[n][:, k, :],
                            start=(k == 0),
                            stop=(k == NK - 1),
                        )
                        mm.ins.ldweights = False
                        if k == NK - 1:
                            mm.then_inc(s_pe, 1)

            @block.gpsimd
            def _(gpsimd):
                # GpSimd can't touch PSUM on TRN2 — it issues the coef SWDGE
                # load and does b0's (SBUF-only) bias add. No terminal s_out
                # wait: the walrus exit sequence (~6us semaphore storm) runs
                # after the last body op, far longer than the 64KB output
                # transfers need to land.
                gpsimd.dma_start(coef_t[:], coefv).then_inc(s_cf, 16)
                gpsimd.wait_ge(s_bcopy, 2)
                gpsimd.wait_ge(s_r0, NCH)
                nc.gpsimd.tensor_add(
                    outf[0][:], out_sb[0][:], bias_sb[0][:]
                ).then_inc(s_addg, 1)

            @block.vector
            def _(vector):
                # Vector owns every PSUM-reading stage-2 op (gpsimd can't).
                # Per chunk pair: 2 mults + 2 strided rank-reduces (~2.75us)
                # inside the ~3.46us closure interval.
                vector.memset(warm_t[:], 0.25).then_inc(s_warm, 1)
                vector.wait_ge(s_cf, 16)
                coef_b = [
                    coef_t[:, b, :].rearrange("p (one r) -> p one r", one=1)
                    for b in range(NB)
                ]
                for n in range(NCH):
                    vector.wait_ge(s_pe, 2 * n + 1)
                    nc.vector.tensor_mul(
                        tmp_g[0][:],
                        pss[2 * n][:].rearrange("p (o r) -> p o r", r=RANK),
                        coef_b[0].to_broadcast((P, OCH, RANK)),
                    )
                    nc.vector.tensor_reduce(
                        out_sb[0][:, n * OCH:(n + 1) * OCH],
                        tmp_g[0][:],
                        axis=mybir.AxisListType.X,
                        op=mybir.AluOpType.add,
                    ).then_inc(s_r0, 1)
                    vector.wait_ge(s_pe, 2 * n + 2)
                    nc.vector.tensor_mul(
                        tmp_v[:],
                        pss[2 * n + 1][:].rearrange("p (o r) -> p o r", r=RANK),
                        coef_b[1].to_broadcast((P, OCH, RANK)),
                    )
                    nc.vector.tensor_reduce(
                        out_sb[1][:, n * OCH:(n + 1) * OCH],
                        tmp_v[:],
                        axis=mybir.AxisListType.X,
                        op=mybir.AluOpType.add,
                    )
                vector.wait_ge(s_bcopy, 2)
                nc.vector.tensor_add(
                    outf[1][:], out_sb[1][:], bias_sb[1][:]
                ).then_inc(s_addv, 1)

    nc.compile()
    return nc


def build_nc_tile(dt_name=DT_NAME):
    dt, _ = _DT_MAP[dt_name]
    f32 = mybir.dt.float32
    # Bacc (not raw Bass): its compile() runs generate_event_semaphores,
    # which splits multi-wait sync_info into EventSemaphore prefixes —
    # walrus accepts at most one wait per regular instruction.
    nc = bacc.Bacc("TRN2", target_bir_lowering=False, debug=False)

    xT = nc.declare_dram_parameter("xT", [IN, B], dt, isOutput=False)
    # w2[n, p, k*CH+c] = W2[k*128+p, n*CH+c]: pre-swizzled on host so each
    # SBUF partition's data is one contiguous 8KB run in DRAM (full-rate DMA).
    w2 = nc.declare_dram_parameter("w2", [NCH, P, NK * CH], dt, isOutput=False)
    coef = nc.declare_dram_parameter("coef", [B, RANK], f32, isOutput=False)
    coefT = nc.declare_dram_parameter("coefT", [RANK, B], dt, isOutput=False)
    biasT = nc.declare_dram_parameter("biasT", [RANK, OUTL], dt, isOutput=False)
    out = nc.declare_dram_parameter("out", [B, OUTL], f32, isOutput=True)

    with tile.TileContext(nc) as tc, ExitStack() as ctx:
        cpool = ctx.enter_context(tc.tile_pool(name="const", bufs=1))
        wpool = ctx.enter_context(tc.tile_pool(name="w", bufs=NCH))
        ppool = ctx.enter_context(tc.tile_pool(name="proj", bufs=6, space="PSUM"))
        bpool = ctx.enter_context(tc.tile_pool(name="biasps", bufs=2, space="PSUM"))
        spool = ctx.enter_context(tc.tile_pool(name="stage2", bufs=4))
        opool = ctx.enter_context(tc.tile_pool(name="outp", bufs=2))

        # Weight tiles for every n-chunk (issued first; n=0 split so the
        # first matmuls can start after only 256KB has landed).
        wts = [wpool.tile([P, NK, CH], dt, tag="w", name=f"wt{n}")
               for n in range(NCH)]
        w2v = w2.rearrange("n p (k c) -> n p k c", c=CH)
        nc.sync.dma_start(wts[0][:, 0:2, :], w2v[0][:, 0:2, :])
        # Full inputT, split in halves (first matmuls need only low k).
        xT_t = cpool.tile([P, NK, B], dt, tag="xT")
        xTv = xT.rearrange("(k p) b -> p k b", p=P)
        nc.sync.dma_start(xT_t[:, 0:NK // 2, :], xTv[:, 0:NK // 2, :])
        nc.sync.dma_start(wts[0][:, 2:NK, :], w2v[0][:, 2:NK, :])
        nc.sync.dma_start(xT_t[:, NK // 2:, :], xTv[:, NK // 2:, :])
        for n in range(1, NCH):
            nc.sync.dma_start(wts[n][:], w2[n].rearrange("p (k c) -> p k c", c=CH))
        coef_t = cpool.tile([P, NB, RANK], f32, tag="coef")
        nc.sync.dma_start(coef_t[:], coef.rearrange("(nb p) r -> p nb r", p=P))
        coefT_t = cpool.tile([RANK, B], dt, tag="coefT")
        nc.sync.dma_start(coefT_t[:], coefT[:])
        biasT_t = cpool.tile([RANK, OUTL], dt, tag="biasT")
        nc.sync.dma_start(biasT_t[:], biasT[:])

        # Bias term: out_bias[b,o] = sum_r coef[b,r] * bias[o,r]
        bias_ps = []
        for b in range(NB):
            bp = bpool.tile([P, OUTL], f32, tag="bias")
            nc.tensor.matmul(
                bp[:], lhsT=coefT_t[:, b * P:(b + 1) * P], rhs=biasT_t[:],
                start=True, stop=True,
            )
            bias_ps.append(bp)

        out_sb = [
            opool.tile([P, OUTL], f32, tag="osum", name=f"osum{b}")
            for b in range(NB)
        ]

        for n in range(NCH):
            pss = [
                ppool.tile([P, CH], f32, tag="proj", name=f"proj{n}_{b}")
                for b in range(NB)
            ]
            wt = wts[n]
            for k in range(NK):
                for b in range(NB):
                    nc.tensor.matmul(
                        pss[b][:],
                        lhsT=xT_t[:, k, b * P:(b + 1) * P],
                        rhs=wt[:, k, :],
                        start=(k == 0),
                        stop=(k == NK - 1),
                    )
            # Rank contraction: multiply by per-(b,r) coef, reduce over r.
            for b in range(NB):
                tmp = spool.tile([P, CH], f32, tag="tmp")
                coef_b = coef_t[:, b, :].rearrange("p (one r) -> p one r", one=1)
                nc.vector.tensor_mul(
                    tmp[:].rearrange("p (o r) -> p o r", r=RANK),
                    pss[b][:].rearrange("p (o r) -> p o r", r=RANK),
                    coef_b.to_broadcast((P, OCH, RANK)),
                )
                nc.vector.tensor_reduce(
                    out_sb[b][:, n * OCH:(n + 1) * OCH],
                    tmp[:].rearrange("p (o r) -> p o r", r=RANK),
                    axis=mybir.AxisListType.X,
                    op=mybir.AluOpType.add,
                )

        for b in range(NB):
            outf = opool.tile([P, OUTL], f32, tag="outf")
            nc.vector.tensor_add(outf[:], out_sb[b][:], bias_ps[b][:])
            nc.sync.dma_start(out[b * P:(b + 1) * P, :], outf[:])

    nc.compile()
    return nc


def prepare_in_maps(input, coef, weight, bias, dt_name=DT_NAME, impl=None):
    impl = impl or IMPL
    _, npdt = _DT_MAP[dt_name]
    xT = np.ascontiguousarray(input.T).astype(npdt)          # (IN, B)
    coefT = np.ascontiguousarray(coef.T).astype(npdt)        # (RANK, B)
    coef32 = np.ascontiguousarray(coef.astype(np.float32))   # (B, RANK)
    in_maps = []
    for c in range(NCORES):
        wsh = weight[c * OUTL:(c + 1) * OUTL]                # (OUTL, IN, RANK)
        # W2[i, o*RANK+r] = wsh[o, i, r]; n-major 512-col chunks; then swizzle
        # (n, i=k*128+p, c) -> (n, p, k, c) so each partition reads one
        # contiguous 8KB run per n-chunk DMA.
        w2 = wsh.transpose(1, 0, 2).reshape(IN, OUTL * RANK)
        w2 = w2.reshape(NK, P, NCH, CH).transpose(2, 1, 0, 3)
        w2 = np.ascontiguousarray(w2.reshape(NCH, P, NK * CH)).astype(npdt)
        biasT = np.ascontiguousarray(
            bias[c * OUTL:(c + 1) * OUTL].T
        ).astype(npdt)                                       # (RANK, OUTL)
        if impl == "v2":
            cb = np.ascontiguousarray(
                np.concatenate([coefT, biasT], axis=1)
            ).astype(npdt)                                   # (RANK, B+OUTL)
            in_maps.append({
                "xT": xT, "w2": w2, "cb": cb, "coef": coef32,
            })
        else:
            in_maps.append({
                "xT": xT, "w2": w2, "coef": coef32,
                "coefT": coefT, "biasT": biasT,
            })
    return in_maps


_NC_CACHE = {}


def _ensure_ntff_hook():
    """The agent image's antenv lacks axon_hooks; inject it and register
    the ctypes NTFF profile hook so trace=True works under axon."""
    import types
    import antenv
    try:
        from antenv import axon_hooks  # noqa: F401
        return
    except ImportError:
        pass
    mod = types.ModuleType("antenv.axon_hooks")
    _state = {"hook": None}
    mod.set_axon_ntff_profile_hook = lambda h: _state.__setitem__("hook", h)
    mod.get_axon_ntff_profile_hook = lambda: _state["hook"]
    sys.modules["antenv.axon_hooks"] = mod
    antenv.axon_hooks = mod
    try:
        from trn_agent_boot.trn_boot import _ntff_profile_via_ctypes
        mod.set_axon_ntff_profile_hook(
            _ntff_profile_via_ctypes("/opt/axon/libaxon_pjrt.so")
        )
    except Exception:
        pass


def build_nc(dt_name=DT_NAME, impl=None):
    impl = impl or IMPL
    if impl == "v2":
        return build_nc_v2(dt_name)
    if impl == "raw":
        return build_nc_raw(dt_name)
    return build_nc_tile(dt_name)


def run(inputs, trace=False, dt_name=DT_NAME, impl=None, **kwargs):
    if trace:
        _ensure_ntff_hook()
    impl = impl or IMPL
    key = (dt_name, impl)
    if key not in _NC_CACHE:
        _NC_CACHE[key] = build_nc(dt_name, impl)
    nc = _NC_CACHE[key]
    in_maps = prepare_in_maps(
        np.asarray(inputs["input"], dtype=np.float32),
        np.asarray(inputs["coef"], dtype=np.float32),
        np.asarray(inputs["weight"], dtype=np.float32),
        np.asarray(inputs["bias"], dtype=np.float32),
        dt_name,
        impl,
    )
    br = run_bass_kernel_spmd(
        nc, in_maps, list(range(NCORES)), trace=trace, **kwargs
    )
    full = np.concatenate(
        [br.results[c]["out"] for c in range(NCORES)], axis=1
    ).astype(np.float32)
    return full, br


def kernel(**inputs):
    full, _ = run(inputs)
    return full



# revision 39
# speedup vs baseline: 1.0179x; 1.0179x over previous
"""MixtureLinear Trainium2 kernel.

Computes, for B=256, IN=1024, OUT=1024, RANK=16:
    out[b,o] = sum_i input[b,i] * sum_r weight[o,i,r] * coef[b,r]
             + sum_r bias[o,r] * coef[b,r]

Strategy (8 NeuronCores, tensor-parallel on OUT):
  - Core c owns OUT rows [128c, 128c+128). It reads only its weight shard
    (1/8 of the 64MB weight tensor), input/coef replicated.
  - Stage 1 (PE): proj[b,(o,r)] = inputT.T @ W2 where W2[i, o*16+r] =
    weight[o,i,r]; K=IN accumulated over 8 psum matmuls per 512-column
    chunk (one psum bank, 32 o's x 16 r's per chunk).
  - Stage 2 (DVE): out[b,o] = sum_r proj[b,(o,r)] * coef[b,r] via a
    broadcast-AP multiply + strided reduce over the innermost rank axis.
  - Bias: one tiny K=16 matmul per b-chunk: coefT.T @ biasT -> psum,
    added in the final DVE add before the output DMA.

Matmul dtype is selectable via MIXL_DT (float16 default; bfloat16 /
float32r / float32 supported). Host pre-casts and pre-transposes shards;
stage-2 and all accumulation stay fp32.
"""

import os
import sys
from contextlib import ExitStack

sys.path.insert(0, "/opt/trn_rl_repo")

import numpy as np
import ml_dtypes

import concourse.bass as bass
import concourse.tile as tile
from concourse import bacc, mybir
from concourse.bass_utils import run_bass_kernel_spmd

B, IN, OUT, RANK = 256, 1024, 1024, 16
NCORES = 8
OUTL = OUT // NCORES        # 128 out rows per core
P = 128                     # partitions
NB = B // P                 # 2 batch chunks
NK = IN // P                # 8 contraction chunks
CH = 512                    # psum chunk: one fp32 bank
NCH = OUTL * RANK // CH     # 4 column chunks per core
OCH = CH // RANK            # 32 o's per chunk

DT_NAME = os.environ.get("MIXL_DT", "float16")
IMPL = os.environ.get("MIXL_IMPL", "v2")
N_WARM1 = int(os.environ.get("MIXL_W1", "22"))
N_WARM2 = int(os.environ.get("MIXL_W2", "0"))
FP8 = os.environ.get("MIXL_FP8", "0") == "1"
NK16 = 6 if FP8 else NK      # k-chunks carried in fp16

_DT_MAP = {
    "float16": (mybir.dt.float16, np.float16),
    "bfloat16": (mybir.dt.bfloat16, ml_dtypes.bfloat16),
    "float32r": (mybir.dt.float32r, np.float32),
    "float32": (mybir.dt.float32, np.float32),
}

_SEMMAX = os.environ.get("MIXL_SEMMAX")
if _SEMMAX:
    # Experiment: cap walrus's semaphore allocation; if its exit-sequence
    # zero-storm length follows the cap, the ~6.8us exit shrinks.
    from concourse import bass_utils as _bu
    _orig_gwa = _bu.get_walrus_args

    def _gwa(*a, **kw):
        return _orig_gwa(*a, **kw) + [f"--max-sem-num={_SEMMAX}"]

    _bu.get_walrus_args = _gwa


class _NoBarrierBlock(bass.BassBlock):
    """BassBlock without the exit drain + all-engine barrier.

    The NRT epilogue (per-engine semaphore-zero storm + exit rendezvous
    chain, ~7us total) runs after each engine's stream ends. With the
    stock barrier, every engine waits for the slowest one before starting
    its epilogue share; without it, early-finishing engines overlap their
    epilogue with the critical-path tail. Output completion is still
    guaranteed: gpsimd's terminal s_out wait orders NEFF completion after
    the output DMAs.
    """

    def __exit__(self, exc_type, exc_val, exc_tb):
        if exc_type is not None:
            return
        for engine, last_body in self.last_body.items():
            with self.bass.body(
                last_body, parent=self.bass.cur_bb, allow_existing_parent=True
            ):
                engine.br(self.end_bb)
        self.bass.switch_bb(self.end_bb)


def build_nc_raw(dt_name=DT_NAME):
    """Raw-Bass (manual Block + semaphores) implementation.

    Design notes (all HW-measured on this container):
    - The NRT execution envelope (entry rendezvous chain + per-engine
      instruction-table loads at the front; per-engine semaphore-zero
      storm + exit rendezvous at the back) costs ~14us on an empty
      kernel. The exit part runs after each ENGINE's stream ends, so the
      Block-end all-engine barrier is deliberately skipped (engines that
      finish early overlap their epilogue with the critical-path tail).
    - A single DMA transfer only sustains ~100-200 GB/s; aggregate tops
      out ~430 GB/s, and the SDMA engines round-robin across everything
      queued at packet granularity. So loads go out in consumption order
      as ~10 chunks with a sliding window of 3 in flight, with tiny
      first chunks so the first matmul can start ASAP.
    - 8 dummy matmuls on memset data warm the PE HAM clock (1.2->2.4GHz
      after ~3.4us of sustained activity) while the first loads land.
    - LDWEIGHTS is emitted separately from a non-self-loading Matmult
      (inst.ldweights=False) so weight loads pipeline into the PE's
      background buffer; fused matmuls measured ~600ns vs ~380ns split.
    """
    dt, _ = _DT_MAP[dt_name]
    f32 = mybir.dt.float32
    f16 = mybir.dt.float16
    nc = bacc.Bacc("TRN2", target_bir_lowering=False, debug=False)

    xT = nc.declare_dram_parameter("xT", [IN, B], dt, isOutput=False)
    w2 = nc.declare_dram_parameter("w2", [NCH, P, NK * CH], dt, isOutput=False)
    coef = nc.declare_dram_parameter("coef", [B, RANK], f32, isOutput=False)
    coefT = nc.declare_dram_parameter("coefT", [RANK, B], dt, isOutput=False)
    biasT = nc.declare_dram_parameter("biasT", [RANK, OUTL], dt, isOutput=False)
    out = nc.declare_dram_parameter("out", [B, OUTL], f32, isOutput=True)

    w2v = w2.rearrange("n p (k c) -> n p k c", c=CH)
    xTv = xT.rearrange("(k p) b -> p k b", p=P)
    coefv = coef.rearrange("(nb p) r -> p nb r", p=P)

    with ExitStack() as ctx:
        sb = lambda shape, d, name: ctx.enter_context(
            nc.sbuf_tensor(name, shape, d))
        xT_t = sb([P, NK, B], dt, "xT_t")
        wts = [sb([P, NK, CH], dt, f"wt{n}") for n in range(NCH)]
        coef_t = sb([P, NB, RANK], f32, "coef_t")
        coefT_t = sb([RANK, B], dt, "coefT_t")
        biasT_t = sb([RANK, OUTL], dt, "biasT_t")
        warm_t = sb([P, CH], dt, "warm_t")
        tmps = [sb([P, OCH, RANK], f16, f"tmp{i}") for i in range(2)]
        out_sb = [sb([P, OUTL], f32, f"osum{b}") for b in range(NB)]
        outf = [sb([P, OUTL], f32, f"outf{b}") for b in range(NB)]
        pss = [ctx.enter_context(nc.psum_tensor(f"ps{g}", [P, CH], f32))
               for g in range(8)]

        # One semaphore per DMA: +16 increments from different transfers
        # interleave (per-SDMA-engine +1s), so aggregate thresholds on a
        # shared sem do not prove any single transfer completed.
        nsem = lambda name: ctx.enter_context(nc.semaphore(name))
        s_x0 = nsem("s_x0")        # xT k=0 (64KB)
        s_w00 = nsem("s_w00")      # w n0 k=0 (128KB)
        s_x13 = nsem("s_x13")      # xT k=1..3
        s_w013 = nsem("s_w013")    # w n0 k=1..3
        s_x47 = nsem("s_x47")      # xT k=4..7
        s_w047 = nsem("s_w047")    # w n0 k=4..7
        s_wn = [nsem(f"s_wn{n}") for n in range(1, NCH)]   # w1..w3 (1MB)
        s_gc = nsem("s_gc")        # coefT
        s_gb = nsem("s_gb")        # biasT
        s_gf = nsem("s_gf")        # coef (fp32)
        s_warm = nsem("s_warm")    # warm-up tile memset
        s_pe = nsem("s_pe")        # psum groups done
        s_dvm = nsem("s_dvm")      # psum mults done
        s_red = nsem("s_red")      # reduces done
        s_dve = nsem("s_dve")      # outf ready
        s_out = nsem("s_out")      # output DMA done

        with _NoBarrierBlock(nc, f"block_{nc.next_id()}") as block:

            @block.sync
            def _(sync):
                xfers = [
                    (xT_t[:, 0:1, :], xTv[:, 0:1, :], s_x0),
                    (wts[0][:, 0:1, :], w2v[0][:, 0:1, :], s_w00),
                    (xT_t[:, 1:4, :], xTv[:, 1:4, :], s_x13),
                    (wts[0][:, 1:4, :], w2v[0][:, 1:4, :], s_w013),
                    (xT_t[:, 4:, :], xTv[:, 4:, :], s_x47),
                    (wts[0][:, 4:, :], w2v[0][:, 4:, :], s_w047),
                ] + [(wts[n][:], w2v[n], s_wn[n - 1]) for n in range(1, NCH)]
                for i, (dst, srcv, sem) in enumerate(xfers):
                    if i >= 3:
                        sync.wait_ge(xfers[i - 3][2], 16)
                    sync.dma_start(dst, srcv).then_inc(sem, 16)

            @block.scalar
            def _(scalar):
                # Output DMAs on the (otherwise idle) ACT ring.
                for b in range(NB):
                    scalar.wait_ge(s_dve, b + 1)
                    scalar.dma_start(out[b * P:(b + 1) * P, :],
                                     outf[b][:]).then_inc(s_out, 16)

            @block.gpsimd
            def _(gpsimd):
                gpsimd.memset(warm_t[:], 0.25).then_inc(s_warm, 1)
                gpsimd.dma_start(coef_t[:], coefv).then_inc(s_gf, 16)
                gpsimd.dma_start(coefT_t[:], coefT[:]).then_inc(s_gc, 16)
                gpsimd.dma_start(biasT_t[:], biasT[:]).then_inc(s_gb, 16)
                # Terminal waiter: holds the Pool stream until outputs are
                # in DRAM, so NEFF completion implies outputs landed.
                gpsimd.wait_ge(s_out, 32)

            @block.tensor
            def _(pe):
                # HAM warm-up: ~8 dummy matmuls on memset data while the
                # first real chunks are still in flight.
                pe.wait_ge(s_warm, 1)
                for _i in range(8):
                    nc.tensor.matmul(pss[2][:], lhsT=warm_t[:, 0:P],
                                     rhs=warm_t[:], start=True, stop=True)
                for n in range(NCH):
                    bank = (2 * n) % 6
                    for k in range(NK):
                        if n == 0:
                            if k == 0:
                                pe.wait_ge(s_x0, 16)
                                pe.wait_ge(s_w00, 16)
                            elif k == 1:
                                pe.wait_ge(s_x13, 16)
                                pe.wait_ge(s_w013, 16)
                            elif k == 4:
                                pe.wait_ge(s_x47, 16)
                                pe.wait_ge(s_w047, 16)
                        elif k == 0:
                            pe.wait_ge(s_wn[n - 1], 16)
                        if n == 3 and k == 0:
                            # banks 0/1 reused: n0 multiplies must be done
                            pe.wait_ge(s_dvm, 2)
                        for b in range(NB):
                            # split LDWEIGHTS + non-self-loading matmul
                            nc.tensor.ldweights(xT_t[:, k, b * P:(b + 1) * P])
                            mm = nc.tensor.matmul(
                                pss[bank + b][:],
                                lhsT=xT_t[:, k, b * P:(b + 1) * P],
                                rhs=wts[n][:, k, :],
                                start=(k == 0),
                                stop=(k == NK - 1),
                            )
                            mm.ins.ldweights = False
                            if k == NK - 1:
                                mm.then_inc(s_pe, 1)
                    if n == 0:
                        # Bias matmuls into dedicated banks 6/7, slotted here
                        # so their input DMAs are long done and the PE stream
                        # never stalls on them.
                        pe.wait_ge(s_gc, 16)
                        pe.wait_ge(s_gb, 16)
                        for b in range(NB):
                            nc.tensor.matmul(
                                pss[6 + b][:, 0:OUTL],
                                lhsT=coefT_t[:, b * P:(b + 1) * P],
                                rhs=biasT_t[:],
                                start=True, stop=True,
                            ).then_inc(s_pe, 1)

            @block.vector
            def _(vector):
                vector.wait_ge(s_gf, 16)
                # s_pe increment order: n0b0=1 n0b1=2 bias0=3 bias1=4
                # n1b0=5 n1b1=6 n2b0=7 n2b1=8 n3b0=9 n3b1=10
                pe_val = {0: (1, 2), 1: (5, 6), 2: (7, 8), 3: (9, 10)}
                g = 0
                for n in range(NCH):
                    bank = (2 * n) % 6
                    for b in range(NB):
                        g += 1
                        vector.wait_ge(s_pe, pe_val[n][b])
                        coef_b = coef_t[:, b, :].rearrange(
                            "p (one r) -> p one r", one=1)
                        tmp = tmps[g % 2]
                        nc.vector.tensor_mul(
                            tmp[:],
                            pss[bank + b][:].rearrange("p (o r) -> p o r", r=RANK),
                            coef_b.to_broadcast((P, OCH, RANK)),
                        ).then_inc(s_dvm, 1)
                        vector.wait_ge(s_dvm, g)
                        nc.vector.tensor_reduce(
                            out_sb[b][:, n * OCH:(n + 1) * OCH],
                            tmp[:],
                            axis=mybir.AxisListType.X,
                            op=mybir.AluOpType.add,
                        ).then_inc(s_red, 1)
                for b in range(NB):
                    vector.wait_ge(s_pe, 3 + b)
                    # all four reduces of this b-chunk (g = b+1, b+3, b+5, b+7)
                    vector.wait_ge(s_red, NB * NCH - NB + b + 1)
                    nc.vector.tensor_add(
                        outf[b][:], out_sb[b][:], pss[6 + b][:, 0:OUTL]
                    ).then_inc(s_dve, 1)

    nc.compile()
    return nc


def build_nc_v2(dt_name=DT_NAME, n_warm1=None, n_warm2=None):
    """v2 raw-Bass implementation, rebuilt from baseline trace analysis.

    Trace findings the baseline left on the table:
    - Steady-state matmuls issue every 216ns (512 cols @ 2.4GHz) — the PE
      pipe itself is perfect. But two ~2us weight-DMA stalls early in the
      stream made the HAM drop the PE clock to 1.2GHz for ~7us.
    - All big loads went through ONE hw queue (sync) at an effective
      ~160GB/s early / 276GB/s late; per-transfer completion order plus a
      window-of-3 starved the leading edge.
    - The stage-2 (rank-contract) chain ran entirely on DVE, serializing
      ~3.1us after the last matmul.
    - Exit = rendezvous + 256-semaphore zero storm (~7us, walrus-fixed).

    v2 changes:
    - Weights stream as 16 x 256KB transfers (k-pair granularity)
      alternated across BOTH hw queues (sync + scalar) in consumption
      order; x as 4 x 128KB split across both; no windowing — hw queues
      process descriptors in order. coef (fp32, stage-2) rides SWDGE.
    - PE: 8 psum banks = 8 (n,b) accumulation groups, no bank reuse, no
      mid-stream back-pressure waits. Chunk waits land one-per-instruction
      (x-waits on ldweights, w-waits on matmuls) so they cost nothing.
    - Bias matmuls run right after warmup into banks 6/7 (n3's banks);
      gpsimd copies them to SBUF long before n3's k0 start=True wipe.
    - Stage-2 split: gpsimd owns b0's mult+reduce+add chain, vector owns
      b1's. Within an engine program order replaces semaphores.
    - Output DMAs on scalar (its weight share is long done); gpsimd's
      terminal s_out wait still orders NEFF completion after the output.
    """
    if n_warm1 is None:
        n_warm1 = N_WARM1
    if n_warm2 is None:
        n_warm2 = N_WARM2
    dt, _ = _DT_MAP[dt_name]
    f32 = mybir.dt.float32
    f16 = mybir.dt.float16
    f8 = mybir.dt.float8e4
    nc = bacc.Bacc("TRN2", target_bir_lowering=False, debug=False)

    xT = nc.declare_dram_parameter("xT", [NK16 * P, B], dt, isOutput=False)
    w2 = nc.declare_dram_parameter("w2", [NCH, P, NK16 * CH], dt,
                                   isOutput=False)
    cb = nc.declare_dram_parameter("cb", [RANK, B + OUTL], dt, isOutput=False)
    coef = nc.declare_dram_parameter("coef", [B, RANK], f32, isOutput=False)
    if FP8:
        x8 = nc.declare_dram_parameter("x8", [2 * P, B], f8, isOutput=False)
        xr8 = nc.declare_dram_parameter("xr8", [2 * P, B], f8, isOutput=False)
        w8 = nc.declare_dram_parameter("w8", [NCH, P, 2 * CH], f8,
                                       isOutput=False)
    out = nc.declare_dram_parameter("out", [B, OUTL], f32, isOutput=True)

    w2v = w2.rearrange("n p (k c) -> n p k c", c=CH)
    xTv = xT.rearrange("(k p) b -> p k b", p=P)
    coefv = coef.rearrange("(nb p) r -> p nb r", p=P)
    if FP8:
        x8v = x8.rearrange("(i p) b -> p i b", p=P)
        xr8v = xr8.rearrange("(i p) b -> p i b", p=P)
        w8v = w8.rearrange("n p (i c) -> n p i c", c=CH)

    with ExitStack() as ctx:
        sb = lambda shape, d, name: ctx.enter_context(
            nc.sbuf_tensor(name, shape, d))
        xT_t = sb([P, NK16, B], dt, "xT_t")
        wts = [sb([P, NK16, CH], dt, f"wt{n}") for n in range(NCH)]
        if FP8:
            x8_t = sb([P, 2, B], f8, "x8_t")
            xr8_t = sb([P, 2, B], f8, "xr8_t")
            w8ts = [sb([P, 2, CH], f8, f"w8t{n}") for n in range(NCH)]
        cb_t = sb([RANK, B + OUTL], dt, "cb_t")
        coef_t = sb([P, NB, RANK], f32, "coef_t")
        warm_t = sb([P, CH], dt, "warm_t")
        tmp_g = [sb([P, OCH, RANK], f16, f"tmp_g{i}") for i in range(2)]
        tmp_v = sb([P, OCH, RANK], f16, "tmp_v")
        bias_sb = [sb([P, OUTL], f32, f"bias_sb{b}") for b in range(NB)]
        out_sb = [sb([P, OUTL], f32, f"osum{b}") for b in range(NB)]
        outf = [sb([P, OUTL], f32, f"outf{b}") for b in range(NB)]
        pss = [ctx.enter_context(nc.psum_tensor(f"ps{g}", [P, CH], f32))
               for g in range(8)]

        nsem = lambda name: ctx.enter_context(nc.semaphore(name))
        s_warm = nsem("s_warm")
        s_cb = nsem("s_cb")
        s_cf = nsem("s_cf")
        s_biasmm = nsem("s_biasmm")
        s_bcopy = nsem("s_bcopy")
        s_x = [nsem(f"s_x{i}") for i in range(NK16 // 2)]  # x k-pairs
        s_w = [[nsem(f"s_w{n}_{kk}") for kk in range(NK16 // 2)]
               for n in range(NCH)]                        # w (n, k-pair)
        if FP8:
            s_x8 = nsem("s_x8")
            s_xr8 = nsem("s_xr8")
            s_w8 = [nsem(f"s_w8_{n}") for n in range(NCH)]
        s_pe = nsem("s_pe")
        s_r0 = nsem("s_r0")      # vector b0 reduces done
        s_addg = nsem("s_addg")
        s_addv = nsem("s_addv")
        s_out = nsem("s_out")

        with _NoBarrierBlock(nc, f"block_{nc.next_id()}") as block:

            def wxfer(eng, n, kk):
                eng.dma_start(
                    wts[n][:, 2 * kk:2 * kk + 2, :],
                    w2v[n][:, 2 * kk:2 * kk + 2, :],
                ).then_inc(s_w[n][kk], 16)

            def xxfer(eng, kk):
                eng.dma_start(
                    xT_t[:, 2 * kk:2 * kk + 2, :],
                    xTv[:, 2 * kk:2 * kk + 2, :],
                ).then_inc(s_x[kk], 16)

            @block.sync
            def _(sync):
                # The 16-SDMA pool is shared across queues (~320GB/s total);
                # transfers are laid out so the two queues jointly deliver
                # in PE consumption order (sync's queue drains faster, so it
                # carries a bit more).
                wxfer(sync, 0, 0)      # w n0 k01   need s
                xxfer(sync, 1)         # x k23      need s+0.86
                wxfer(sync, 0, 2)      # w n0 k45   need s+1.73
                if FP8:
                    sync.dma_start(w8ts[0][:], w8v[0]).then_inc(s_w8[0], 16)
                else:
                    xxfer(sync, 3)     # x k67      need s+2.6
                for n in range(1, NCH):
                    wxfer(sync, n, 0)
                    wxfer(sync, n, 2)
                    if FP8 and n == 2:
                        sync.dma_start(w8ts[2][:],
                                       w8v[2]).then_inc(s_w8[2], 16)
                # Output b1 rides the (long-idle) sync queue, parallel with
                # scalar's b0 output issue.
                sync.wait_ge(s_addv, 1)
                sync.dma_start(out[P:2 * P, :],
                               outf[1][:]).then_inc(s_out, 16)

            @block.scalar
            def _(scalar):
                scalar.dma_start(cb_t[:], cb[:]).then_inc(s_cb, 16)
                xxfer(scalar, 0)       # x k01      need s
                wxfer(scalar, 0, 1)    # w n0 k23   need s+0.86
                xxfer(scalar, 2)       # x k45      need s+1.73
                if FP8:
                    scalar.dma_start(x8_t[:], x8v).then_inc(s_x8, 16)
                    scalar.dma_start(xr8_t[:], xr8v).then_inc(s_xr8, 16)
                    for n in range(1, NCH):
                        wxfer(scalar, n, 1)
                        if n in (1, 3):
                            scalar.dma_start(w8ts[n][:],
                                             w8v[n]).then_inc(s_w8[n], 16)
                else:
                    wxfer(scalar, 0, 3)   # w n0 k67  need s+2.6
                    for n in range(1, NCH):
                        wxfer(scalar, n, 1)
                        wxfer(scalar, n, 3)
                # Bias psum -> SBUF (gpsimd can't read PSUM; scalar can).
                scalar.wait_ge(s_biasmm, 2)
                for b in range(NB):
                    nc.scalar.copy(
                        bias_sb[b][:], pss[6 + b][:, 0:OUTL]
                    ).then_inc(s_bcopy, 1)
                scalar.wait_ge(s_addg, 1)
                scalar.dma_start(out[0:P, :],
                                 outf[0][:]).then_inc(s_out, 16)

            @block.tensor
            def _(pe):
                # Pure warmup until the first weight chunks are visible; any
                # mid-warmup stall would reset the HAM clock ramp (3us of
                # continuous execution -> 2.4GHz), so nothing here waits on
                # DMA. Bias matmuls run at the n0/n1 seam instead, inside
                # the full-speed stream (cb lands long before).
                pe.wait_ge(s_warm, 1)
                for _i in range(n_warm1 + n_warm2):
                    nc.tensor.matmul(pss[0][:], lhsT=warm_t[:, 0:P],
                                     rhs=warm_t[:], start=True, stop=True)
                # n0-n2: b-interleaved k-loop (spreads the x-arrival deadline
                # over the full n0 phase). n3: b0 k-sweep then b1 k-sweep, so
                # b0's last chunk closes 1.73us before b1's and the stage-2
                # tail pipelines into the matmul stream. With FP8, k-chunks
                # 6-7 run as two e4m3 DoubleRow passes (x8 then the x
                # quantization residual xr8, both against w8) at 0.5
                # cycles/col, closing each group.
                DR = mybir.MatmulPerfMode.DoubleRow

                def fp8_pass(n, b, src_t, stop, wait_sems):
                    for sem, val in wait_sems:
                        pe.wait_ge(sem, val)
                    nc.tensor.ldweights(
                        src_t[:, :, b * P:(b + 1) * P], perf_mode=DR)
                    mm = nc.tensor.matmul(
                        pss[2 * n + b][:],
                        lhsT=src_t[:, :, b * P:(b + 1) * P],
                        rhs=w8ts[n][:],
                        start=False, stop=stop, perf_mode=DR,
                    )
                    mm.ins.ldweights = False
                    if stop:
                        mm.then_inc(s_pe, 1)

                for n in range(NCH):
                    if n == 1:
                        # Bias matmuls into banks 6/7 (n3's banks, reused
                        # much later); ~300ns each inside the hot stream.
                        pe.wait_ge(s_cb, 16)
                        for b in range(NB):
                            nc.tensor.matmul(
                                pss[6 + b][:, 0:OUTL],
                                lhsT=cb_t[:, b * P:(b + 1) * P],
                                rhs=cb_t[:, B:B + OUTL],
                                start=True, stop=True,
                            ).then_inc(s_biasmm, 1)

                    def f16_mm(k, b):
                        if b == 0:
                            if n == 0 and k % 2 == 0:
                                pe.wait_ge(s_x[k // 2], 16)
                            if n == 3 and k == 0:
                                pe.wait_ge(s_bcopy, 2)
                        nc.tensor.ldweights(
                            xT_t[:, k, b * P:(b + 1) * P])
                        if b == 0 and k % 2 == 0:
                            pe.wait_ge(s_w[n][k // 2], 16)
                        mm = nc.tensor.matmul(
                            pss[2 * n + b][:],
                            lhsT=xT_t[:, k, b * P:(b + 1) * P],
                            rhs=wts[n][:, k, :],
                            start=(k == 0),
                            stop=(not FP8 and k == NK16 - 1),
                        )
                        mm.ins.ldweights = False
                        if not FP8 and k == NK16 - 1:
                            mm.then_inc(s_pe, 1)

                    def fp8_tail(b, first_b):
                        ws = []
                        if n == 0 and first_b:
                            ws = [(s_x8, 16)]
                        if first_b:
                            ws.append((s_w8[n], 16))
                        fp8_pass(n, b, x8_t, False, ws)
                        fp8_pass(n, b, xr8_t, True,
                                 [(s_xr8, 16)] if n == 0 and first_b else [])

                    if n < NCH - 1:
                        for k in range(NK16):
                            for b in range(NB):
                                f16_mm(k, b)
                        if FP8:
                            fp8_tail(0, True)
                            fp8_tail(1, False)
                    else:
                        for b in range(NB):
                            for k in range(NK16):
                                f16_mm(k, b)
                            if FP8:
                                fp8_tail(b, b == 0)

            @block.gpsimd
            def _(gpsimd):
                # GpSimd can't touch PSUM on TRN2 — it issues the coef SWDGE
                # load and does b0's (SBUF-only) bias add. No terminal s_out
                # wait: the walrus exit sequence (~6us semaphore storm) runs
                # after the last body op, far longer than the 64KB output
                # transfers need to land.
                gpsimd.dma_start(coef_t[:], coefv).then_inc(s_cf, 16)
                gpsimd.wait_ge(s_bcopy, 2)
                gpsimd.wait_ge(s_r0, NCH)
                nc.gpsimd.tensor_add(
                    outf[0][:], out_sb[0][:], bias_sb[0][:]
                ).then_inc(s_addg, 1)

            @block.vector
            def _(vector):
                # Vector owns every PSUM-reading stage-2 op (gpsimd can't).
                # Per chunk pair: 2 mults + 2 strided rank-reduces (~2.75us)
                # inside the ~3.46us closure interval.
                vector.memset(warm_t[:], 0.25).then_inc(s_warm, 1)
                vector.wait_ge(s_cf, 16)
                coef_b = [
                    coef_t[:, b, :].rearrange("p (one r) -> p one r", one=1)
                    for b in range(NB)
                ]
                for n in range(NCH):
                    vector.wait_ge(s_pe, 2 * n + 1)
                    nc.vector.tensor_mul(
                        tmp_g[0][:],
                        pss[2 * n][:].rearrange("p (o r) -> p o r", r=RANK),
                        coef_b[0].to_broadcast((P, OCH, RANK)),
                    )
                    nc.vector.tensor_reduce(
                        out_sb[0][:, n * OCH:(n + 1) * OCH],
                        tmp_g[0][:],
                        axis=mybir.AxisListType.X,
                        op=mybir.AluOpType.add,
                    ).then_inc(s_r0, 1)
                    vector.wait_ge(s_pe, 2 * n + 2)
                    nc.vector.tensor_mul(
                        tmp_v[:],
                        pss[2 * n + 1][:].rearrange("p (o r) -> p o r", r=RANK),
                        coef_b[1].to_broadcast((P, OCH, RANK)),
                    )
                    nc.vector.tensor_reduce(
                        out_sb[1][:, n * OCH:(n + 1) * OCH],
                        tmp_v[:],
                        axis=mybir.AxisListType.X,
                        op=mybir.AluOpType.add,
                    )
                vector.wait_ge(s_bcopy, 2)
                nc.vector.tensor_add(
                    outf[1][:], out_sb[1][:], bias_sb[1][:]
                ).then_inc(s_addv, 1)

    nc.compile()
    return nc


def build_nc_tile(dt_name=DT_NAME):
    dt, _ = _DT_MAP[dt_name]
    f32 = mybir.dt.float32
    # Bacc (not raw Bass): its compile() runs generate_event_semaphores,
    # which splits multi-wait sync_info into EventSemaphore prefixes —
    # walrus accepts at most one wait per regular instruction.
    nc = bacc.Bacc("TRN2", target_bir_lowering=False, debug=False)

    xT = nc.declare_dram_parameter("xT", [IN, B], dt, isOutput=False)
    # w2[n, p, k*CH+c] = W2[k*128+p, n*CH+c]: pre-swizzled on host so each
    # SBUF partition's data is one contiguous 8KB run in DRAM (full-rate DMA).
    w2 = nc.declare_dram_parameter("w2", [NCH, P, NK * CH], dt, isOutput=False)
    coef = nc.declare_dram_parameter("coef", [B, RANK], f32, isOutput=False)
    coefT = nc.declare_dram_parameter("coefT", [RANK, B], dt, isOutput=False)
    biasT = nc.declare_dram_parameter("biasT", [RANK, OUTL], dt, isOutput=False)
    out = nc.declare_dram_parameter("out", [B, OUTL], f32, isOutput=True)

    with tile.TileContext(nc) as tc, ExitStack() as ctx:
        cpool = ctx.enter_context(tc.tile_pool(name="const", bufs=1))
        wpool = ctx.enter_context(tc.tile_pool(name="w", bufs=NCH))
        ppool = ctx.enter_context(tc.tile_pool(name="proj", bufs=6, space="PSUM"))
        bpool = ctx.enter_context(tc.tile_pool(name="biasps", bufs=2, space="PSUM"))
        spool = ctx.enter_context(tc.tile_pool(name="stage2", bufs=4))
        opool = ctx.enter_context(tc.tile_pool(name="outp", bufs=2))

        # Weight tiles for every n-chunk (issued first; n=0 split so the
        # first matmuls can start after only 256KB has landed).
        wts = [wpool.tile([P, NK, CH], dt, tag="w", name=f"wt{n}")
               for n in range(NCH)]
        w2v = w2.rearrange("n p (k c) -> n p k c", c=CH)
        nc.sync.dma_start(wts[0][:, 0:2, :], w2v[0][:, 0:2, :])
        # Full inputT, split in halves (first matmuls need only low k).
        xT_t = cpool.tile([P, NK, B], dt, tag="xT")
        xTv = xT.rearrange("(k p) b -> p k b", p=P)
        nc.sync.dma_start(xT_t[:, 0:NK // 2, :], xTv[:, 0:NK // 2, :])
        nc.sync.dma_start(wts[0][:, 2:NK, :], w2v[0][:, 2:NK, :])
        nc.sync.dma_start(xT_t[:, NK // 2:, :], xTv[:, NK // 2:, :])
        for n in range(1, NCH):
            nc.sync.dma_start(wts[n][:], w2[n].rearrange("p (k c) -> p k c", c=CH))
        coef_t = cpool.tile([P, NB, RANK], f32, tag="coef")
        nc.sync.dma_start(coef_t[:], coef.rearrange("(nb p) r -> p nb r", p=P))
        coefT_t = cpool.tile([RANK, B], dt, tag="coefT")
        nc.sync.dma_start(coefT_t[:], coefT[:])
        biasT_t = cpool.tile([RANK, OUTL], dt, tag="biasT")
        nc.sync.dma_start(biasT_t[:], biasT[:])

        # Bias term: out_bias[b,o] = sum_r coef[b,r] * bias[o,r]
        bias_ps = []
        for b in range(NB):
            bp = bpool.tile([P, OUTL], f32, tag="bias")
            nc.tensor.matmul(
                bp[:], lhsT=coefT_t[:, b * P:(b + 1) * P], rhs=biasT_t[:],
                start=True, stop=True,
            )
            bias_ps.append(bp)

        out_sb = [
            opool.tile([P, OUTL], f32, tag="osum", name=f"osum{b}")
            for b in range(NB)
        ]

        for n in range(NCH):
            pss = [
                ppool.tile([P, CH], f32, tag="proj", name=f"proj{n}_{b}")
                for b in range(NB)
            ]
            wt = wts[n]
            for k in range(NK):
                for b in range(NB):
                    nc.tensor.matmul(
                        pss[b][:],
                        lhsT=xT_t[:, k, b * P:(b + 1) * P],
                        rhs=wt[:, k, :],
                        start=(k == 0),
                        stop=(k == NK - 1),
                    )
            # Rank contraction: multiply by per-(b,r) coef, reduce over r.
            for b in range(NB):
                tmp = spool.tile([P, CH], f32, tag="tmp")
                coef_b = coef_t[:, b, :].rearrange("p (one r) -> p one r", one=1)
                nc.vector.tensor_mul(
                    tmp[:].rearrange("p (o r) -> p o r", r=RANK),
                    pss[b][:].rearrange("p (o r) -> p o r", r=RANK),
                    coef_b.to_broadcast((P, OCH, RANK)),
                )
                nc.vector.tensor_reduce(
                    out_sb[b][:, n * OCH:(n + 1) * OCH],
                    tmp[:].rearrange("p (o r) -> p o r", r=RANK),
                    axis=mybir.AxisListType.X,
                    op=mybir.AluOpType.add,
                )

        for b in range(NB):
            outf = opool.tile([P, OUTL], f32, tag="outf")
            nc.vector.tensor_add(outf[:], out_sb[b][:], bias_ps[b][:])
            nc.sync.dma_start(out[b * P:(b + 1) * P, :], outf[:])

    nc.compile()
    return nc


def prepare_in_maps(input, coef, weight, bias, dt_name=DT_NAME, impl=None):
    impl = impl or IMPL
    _, npdt = _DT_MAP[dt_name]
    np8 = mybir.dt.np(mybir.dt.float8e4)
    KS = NK16 * P                                            # fp16 k extent
    xT = np.ascontiguousarray(input[:, :KS].T).astype(npdt)  # (KS, B)
    coefT = np.ascontiguousarray(coef.T).astype(npdt)        # (RANK, B)
    coef32 = np.ascontiguousarray(coef.astype(np.float32))   # (B, RANK)
    if FP8 and impl == "v2":
        xa = input[:, KS:].T.astype(np.float32)              # (2P, B)
        x8 = xa.astype(np8)
        xr8 = (xa - x8.astype(np.float32)).astype(np8)
        x8 = np.ascontiguousarray(x8)
        xr8 = np.ascontiguousarray(xr8)
    in_maps = []
    for c in range(NCORES):
        wsh = weight[c * OUTL:(c + 1) * OUTL]                # (OUTL, IN, RANK)
        # W2[i, o*RANK+r] = wsh[o, i, r]; n-major 512-col chunks; then swizzle
        # (n, i=k*128+p, c) -> (n, p, k, c) so each partition reads
        # contiguous per-partition runs per k-slice DMA.
        w2f = wsh.transpose(1, 0, 2).reshape(IN, OUTL * RANK)
        w2f = w2f.reshape(NK, P, NCH, CH).transpose(2, 1, 0, 3)  # (n,p,k,c)
        w2 = np.ascontiguousarray(
            w2f[:, :, :NK16].reshape(NCH, P, NK16 * CH)).astype(npdt)
        biasT = np.ascontiguousarray(
            bias[c * OUTL:(c + 1) * OUTL].T
        ).astype(npdt)                                       # (RANK, OUTL)
        if impl == "v2":
            cb = np.ascontiguousarray(
                np.concatenate([coefT, biasT], axis=1)
            ).astype(npdt)                                   # (RANK, B+OUTL)
            m = {"xT": xT, "w2": w2, "cb": cb, "coef": coef32}
            if FP8:
                m["x8"] = x8
                m["xr8"] = xr8
                m["w8"] = np.ascontiguousarray(
                    w2f[:, :, NK16:].reshape(NCH, P, 2 * CH)
                ).astype(np8)
            in_maps.append(m)
        else:
            in_maps.append({
                "xT": xT, "w2": w2, "coef": coef32,
                "coefT": coefT, "biasT": biasT,
            })
    return in_maps


_NC_CACHE = {}


def _ensure_ntff_hook():
    """The agent image's antenv lacks axon_hooks; inject it and register
    the ctypes NTFF profile hook so trace=True works under axon."""
    import types
    import antenv
    try:
        from antenv import axon_hooks  # noqa: F401
        return
    except ImportError:
        pass
    mod = types.ModuleType("antenv.axon_hooks")
    _state = {"hook": None}
    mod.set_axon_ntff_profile_hook = lambda h: _state.__setitem__("hook", h)
    mod.get_axon_ntff_profile_hook = lambda: _state["hook"]
    sys.modules["antenv.axon_hooks"] = mod
    antenv.axon_hooks = mod
    try:
        from trn_agent_boot.trn_boot import _ntff_profile_via_ctypes
        mod.set_axon_ntff_profile_hook(
            _ntff_profile_via_ctypes("/opt/axon/libaxon_pjrt.so")
        )
    except Exception:
        pass


def build_nc(dt_name=DT_NAME, impl=None):
    impl = impl or IMPL
    if impl == "v2":
        return build_nc_v2(dt_name)
    if impl == "raw":
        return build_nc_raw(dt_name)
    return build_nc_tile(dt_name)


def run(inputs, trace=False, dt_name=DT_NAME, impl=None, **kwargs):
    if trace:
        _ensure_ntff_hook()
    impl = impl or IMPL
    key = (dt_name, impl, FP8)
    if key not in _NC_CACHE:
        _NC_CACHE[key] = build_nc(dt_name, impl)
    nc = _NC_CACHE[key]
    in_maps = prepare_in_maps(
        np.asarray(inputs["input"], dtype=np.float32),
        np.asarray(inputs["coef"], dtype=np.float32),
        np.asarray(inputs["weight"], dtype=np.float32),
        np.asarray(inputs["bias"], dtype=np.float32),
        dt_name,
        impl,
    )
    br = run_bass_kernel_spmd(
        nc, in_maps, list(range(NCORES)), trace=trace, **kwargs
    )
    full = np.concatenate(
        [br.results[c]["out"] for c in range(NCORES)], axis=1
    ).astype(np.float32)
    return full, br


def kernel(**inputs):
    full, _ = run(inputs)
    return full

